# revision 33
# baseline (speedup 1.0000x reference)
"""Trainium2 Bass kernel for AutoRegressiveLSTMEncoder — v5.

Data parallel over 8 NeuronCores (batch 32768 -> 4096/core); feature-on-
partition / batch-on-free layout; full unroll of the 32 steps; h/p state
double-buffered in SBUF (parity by step); c streamed bf16 through HBM.

Design (per step, per core):
  - Recurrent matmuls fp8e4 DoubleRow (K=256/matmul); G0 (step-invariant
    input term) streamed bf16 from HBM and injected into PSUM via identity
    matmuls.
  - Gate PSUM organized as 2-bank [128, 1024] tiles (pairs (i,f) and (g,o)),
    each evicted by ONE wide sigmoid activation.  tanh for the g-gate via
    tanh(x) = 2*sigmoid(2x) - 1 with the x2 folded into g rows on the host;
    the affine fix runs on DVE in fp32 (no cancellation loss).
  - z/softmax phase uses NO exp/ln activations: p ~ softplus(x) with
    x in [-0.6, 0.6], evaluated as the DVE polynomial
    sp(x) = x/2 + ln2 + u/8 - u^2/192  (u = x^2, |err| < 2e-5).
    The whole kernel therefore uses a single activation table set
    (sigmoid/tanh/identity) -> zero LoadActFuncSet swaps in steady state.
  - h (fp8 DoubleRow layout) and p (bf16) are double-buffered by step parity:
    writes of step t target the buffer read by step t+1, so no WAR/RAW chain
    ever stalls the PE stream inside a step.
  - z work for tile n is emitted interleaved into tile n+1's gate stream
    (DR matmuls at r==1, poly at r==2, sum/recip/out at r==4) so the PE is
    never head-of-line blocked on the softmax chain.
  - DMA: all on the SP HWDGE queue; g0/c loads for tile n+1 are emitted one
    iteration ahead of tile n's c stores (rolling prefetch, ~1 tile deep).

Numerics: weights carry a x64 scale (x128 for g rows) folded on the host for
fp8 range; activations descale with scale=1/64. h stored fp8 (plain scale),
c round-trips bf16, cell combine in fp32.
"""

import sys

sys.path.insert(0, "/opt/trn_rl_repo")

import numpy as np
import ml_dtypes
from contextlib import ExitStack

import concourse.bass as bass
import concourse.bacc as bacc
import concourse.tile as tile
from concourse import mybir

AF = mybir.ActivationFunctionType
DT = mybir.dt
ALU = mybir.AluOpType

B, E, D, A, H = 32768, 300, 32, 64, 1024
G4 = 4 * H
NCORES = 8
BL = B // NCORES  # 4096
NT = 512
NB = BL // NT  # 8
KXP = 384  # E=300 padded to 3*128
SCALE = 64.0
LN2 = 0.6931471805599453


def build_nc(BL=BL, nsteps=D):
    import os as _os

    ABL_G0 = _os.environ.get("ABL_G0", "0") == "1"   # timing ablation: no G0 stream
    ABL_C = _os.environ.get("ABL_C", "0") == "1"     # timing ablation: no c round-trip
    ABL_Z = _os.environ.get("ABL_Z", "0") == "1"     # timing ablation: no z phase
    ABL_CELL = _os.environ.get("ABL_CELL", "0") == "1"  # timing ablation: no cell/tail
    ABL_NOID = _os.environ.get("ABL_NOID", "0") == "1"   # timing: no identity MMs
    ABL_NOWBIG = _os.environ.get("ABL_NOWBIG", "0") == "1"  # timing: no wbig MMs
    ABL_BF16 = _os.environ.get("ABL_BF16", "0") == "1"   # timing: bf16 recurrent MMs
    WBFIRST = _os.environ.get("WBFIRST", "0") == "1"     # wbig before the DR block
    NB = BL // NT
    assert BL == NB * NT

    nc = bacc.Bacc("TRN2", target_bir_lowering=False, debug=False)
    f32, bf, f8 = DT.float32, DT.bfloat16, DT.float8e4
    DR = mybir.MatmulPerfMode.DoubleRow

    # ---- external inputs (host pre-tiled / pre-transposed / pre-cast) ----
    xT = nc.dram_tensor("xT", (3, 128, BL), bf, kind="ExternalInput")
    WxhT = nc.dram_tensor("WxhT", (3, 128, H), bf, kind="ExternalInput")
    bxh = nc.dram_tensor("bxh", (128, 8), f32, kind="ExternalInput")
    WihADR = nc.dram_tensor("WihADR", (4, 128, 2, G4), f8, kind="ExternalInput")
    WbigT = nc.dram_tensor("WbigT", (128, 2, G4), f8, kind="ExternalInput")
    WhhDR = nc.dram_tensor("WhhDR", (4, 128, 2, G4), f8, kind="ExternalInput")
    bg = nc.dram_tensor("bg", (128, 32), f32, kind="ExternalInput")
    WhzDR = nc.dram_tensor("WhzDR", (4, 128, 2, A), f8, kind="ExternalInput")
    ident = nc.dram_tensor("ident", (128, 128), bf, kind="ExternalInput")
    bhz = nc.dram_tensor("bhz", (A, 1), f32, kind="ExternalInput")
    onesB = nc.dram_tensor("onesB", (A, 128), bf, kind="ExternalInput")

    # ---- output ----
    p_all = nc.dram_tensor("p_all", (nsteps, A, BL), f32, kind="ExternalOutput")

    # ---- internal DRAM scratch ----
    G0_d = nc.dram_tensor("G0_d", (NB, 8, 128, 4 * NT), bf, kind="Internal")
    c_d = [
        nc.dram_tensor(f"c_d{i}", (NB, 8, 128, NT), bf, kind="Internal")
        for i in (0, 1)
    ]

    with tile.TileContext(nc) as tc, ExitStack() as ctx:
        # ========== prologue: t_h and G0 (fused per batch tile) ==========
        with ExitStack() as pro:
            wpro = pro.enter_context(tc.tile_pool(name="wpro", bufs=1))
            wxh = [wpro.tile([128, H], bf, tag=f"wxh{k}", name=f"wxh{k}") for k in range(3)]
            wa = [
                wpro.tile([128, 2, G4], f8, tag=f"wa{k}", name=f"wa{k}")
                for k in range(4)
            ]
            bxh_t = wpro.tile([128, 8], f32, tag="bxh")
            bg_t = wpro.tile([128, 32], f32, tag="bg")
            for k in range(3):
                nc.sync.dma_start(wxh[k][:], WxhT[k])
            for k in range(4):
                nc.sync.dma_start(wa[k][:], WihADR[k])
            nc.sync.dma_start(bxh_t[:], bxh[:])
            nc.sync.dma_start(bg_t[:], bg[:])

            pps = pro.enter_context(tc.tile_pool(name="pps", bufs=8, space="PSUM"))
            xr_p = pro.enter_context(tc.tile_pool(name="pxr", bufs=2))
            th_p = pro.enter_context(tc.tile_pool(name="pth", bufs=2))
            g0_p = pro.enter_context(tc.tile_pool(name="pg0", bufs=2))
            for n in range(NB):
                sl = slice(n * NT, (n + 1) * NT)
                xr = [xr_p.tile([128, NT], bf, tag=f"xr{k}", name=f"xr{k}") for k in range(3)]
                for k in range(3):
                    nc.sync.dma_start(xr[k][:], xT[k][:, sl])
                # t_h stored fp8, DoubleRow-interleaved (rhs for the G0 matmuls)
                thdr = [
                    th_p.tile([128, 2, NT], f8, tag=f"th{k}", name=f"th{k}")
                    for k in range(4)
                ]
                for mh in range(8):
                    ps = pps.tile([128, NT], f32, tag="ps")
                    for k in range(3):
                        nc.tensor.matmul(
                            ps[:],
                            wxh[k][:, mh * 128 : (mh + 1) * 128],
                            xr[k][:],
                            start=(k == 0),
                            stop=(k == 2),
                        )
                    nc.scalar.activation(
                        thdr[mh // 2][:, mh % 2, :], ps[:], AF.Tanh,
                        bias=bxh_t[:, mh : mh + 1],
                    )
                for r in range(8):
                    g0t = g0_p.tile([128, 4 * NT], bf, tag="g0t")
                    for gi in range(4):
                        m = gi * 8 + r
                        ps = pps.tile([128, NT], f32, tag="ps")
                        for k in range(4):
                            nc.tensor.matmul(
                                ps[:],
                                wa[k][:, :, m * 128 : (m + 1) * 128],
                                thdr[k][:],
                                start=(k == 0),
                                stop=(k == 3),
                                perf_mode=DR,
                            )
                        nc.scalar.activation(
                            g0t[:, gi * NT : (gi + 1) * NT],
                            ps[:],
                            AF.Identity,
                            bias=bg_t[:, m : m + 1],
                        )
                    nc.sync.dma_start(G0_d[n, r], g0t[:])

        # ========== resident weights + state ==========
        import os as _os2
        _abl_bf16 = _os2.environ.get("ABL_BF16", "0") == "1"
        wres = ctx.enter_context(tc.tile_pool(name="wres", bufs=1))
        whdr = [
            wres.tile([128, 2, G4], f8, tag=f"wh{k}", name=f"wh{k}") for k in range(4)
        ] if not _abl_bf16 else []
        wb_t = wres.tile([128, 2, G4], f8, tag="wbig")
        wzdr = [
            wres.tile([128, 2, A], f8, tag=f"wz{k}", name=f"wz{k}") for k in range(4)
        ]
        id_t = wres.tile([128, 128], bf, tag="ident")
        bhz_t = wres.tile([A, 1], f32, tag="bhz")
        onesB_t = wres.tile([A, 128], bf, tag="onesB")
        for k in range(len(whdr)):
            nc.sync.dma_start(whdr[k][:], WhhDR[k])
        nc.sync.dma_start(wb_t[:], WbigT[:])
        for k in range(4):
            nc.sync.dma_start(wzdr[k][:], WhzDR[k])
        nc.sync.dma_start(id_t[:], ident[:])
        nc.sync.dma_start(bhz_t[:], bhz[:])
        nc.sync.dma_start(onesB_t[:], onesB[:])

        # double-buffered recurrent state (parity by step; t=0 writes [1],
        # never reads [0], so no init needed).
        # hdr2[pb][P][kp, j, b] = h[(2P+j)*128+kp, b]  (DoubleRow interleave)
        hdr2 = [
            [
                wres.tile([128, 2, BL], f8, tag=f"h{pb}_{k}", name=f"h{pb}_{k}")
                for k in range(4)
            ]
            for pb in (0, 1)
        ]
        # p state in fp8 DoubleRow layout: p5[pb][P, j, b] = 8*p[j*32+P, b] for
        # P<32; partitions 32..127 are zero (matching zero weight rows in wb_t)
        p5 = [
            wres.tile([128, 2, BL], f8, tag=f"p{pb}", name=f"p{pb}") for pb in (0, 1)
        ]
        nc.vector.memset(p5[0][:], 0.0)
        nc.vector.memset(p5[1][:], 0.0)

        # ========== main loop pools ==========
        gps = ctx.enter_context(tc.tile_pool(name="gps", bufs=3, space="PSUM"))
        zpp = ctx.enter_context(tc.tile_pool(name="zpp", bufs=1, space="PSUM"))
        smp = ctx.enter_context(tc.tile_pool(name="smp", bufs=1, space="PSUM"))
        g0r_p = ctx.enter_context(tc.tile_pool(name="g0r", bufs=7))
        cin_p = ctx.enter_context(tc.tile_pool(name="cin", bufs=8))
        gs_p = ctx.enter_context(tc.tile_pool(name="gsp", bufs=3))
        cell_p = ctx.enter_context(tc.tile_pool(name="cell", bufs=2))
        z_p = ctx.enter_context(tc.tile_pool(name="zp", bufs=1))

        g0map = {}
        cinmap = {}

        if ABL_G0:
            g0fix = [g0r_p.tile([128, 4 * NT], bf, tag="g0t", name=f"g0f{r}") for r in range(8)]
            for r in range(8):
                nc.sync.dma_start(g0fix[r][:], G0_d[0, r])
        if ABL_C:
            cfix = cin_p.tile([128, NT], bf, tag="cin", name="cfix")
            nc.vector.memset(cfix[:], 0.0)
        if ABL_CELL:
            for pb in (0, 1):
                for k in range(4):
                    nc.vector.memset(hdr2[pb][k][:], 0.0)
        if ABL_BF16:
            whbf = wres.tile([128, G4], bf, tag="whbf")
            hbf = wres.tile([128, BL], bf, tag="hbf")
            nc.vector.memset(whbf[:], 0.01)
            nc.vector.memset(hbf[:], 0.01)

        def load_g0(tt, tn, r):
            if ABL_G0:
                g0map[(tt, tn, r)] = g0fix[r]
                return
            t_ = g0r_p.tile([128, 4 * NT], bf, tag="g0t")
            nc.sync.dma_start(t_[:], G0_d[tn, r])
            g0map[(tt, tn, r)] = t_

        def load_cin(tt, tn, r):
            if ABL_C:
                cinmap[(tt, tn, r)] = cfix
                return
            t_ = cin_p.tile([128, NT], bf, tag="cin")
            nc.sync.dma_start(t_[:], c_d[tt % 2][tn, r])
            cinmap[(tt, tn, r)] = t_

        # initial loads for (t=0, n=0)
        for r in range(8):
            load_g0(0, 0, r)

        # deferred z job: one in flight.  fields: t, n, slz, h (hdr list),
        # pslot, zps, x, q2, q2b
        zjob = None

        def z_dr(job):
            zps = zpp.tile([A, NT], f32, tag="zz")
            for k in range(4):
                nc.tensor.matmul(
                    zps[:], wzdr[k][:], job["h"][k][:, :, job["slz"]],
                    start=(k == 0), stop=(k == 3), perf_mode=DR,
                )
            job["zps"] = zps

        def z_poly(job):
            zps = job["zps"]
            x = z_p.tile([A, NT], f32, tag="zx")
            nc.vector.tensor_scalar(x[:], zps[:], 1.0 / SCALE, bhz_t[:], ALU.mult, ALU.add)
            u = z_p.tile([A, NT], f32, tag="zu")
            nc.vector.tensor_tensor(u[:], x[:], x[:], ALU.mult)
            w = z_p.tile([A, NT], f32, tag="zw")
            nc.vector.tensor_scalar(w[:], u[:], -1.0 / 192.0, 0.125, ALU.mult, ALU.add)
            v = z_p.tile([A, NT], f32, tag="zv")
            nc.vector.tensor_tensor(v[:], w[:], u[:], ALU.mult)
            q2 = z_p.tile([A, NT], f32, tag="zq2")
            nc.vector.scalar_tensor_tensor(q2[:], x[:], 0.5, v[:], ALU.mult, ALU.add)
            q2b = z_p.tile([A, NT], bf, tag="zq2b")
            nc.vector.tensor_scalar(q2b[:], q2[:], LN2, None, ALU.add)
            job["q2"], job["q2b"] = q2, q2b

        def z_flush(job):
            sb = smp.tile([128, NT], f32, tag="sm")
            nc.tensor.matmul(sb[:], onesB_t[:], job["q2b"][:], start=True, stop=True)
            rec = z_p.tile([A, NT], f32, tag="zrec")
            nc.vector.reciprocal(rec[:], sb[:A, :])
            pt = z_p.tile([A, NT], f32, tag="zpt")
            nc.vector.scalar_tensor_tensor(pt[:], job["q2"][:], LN2, rec[:], ALU.add, ALU.mult)
            nc.sync.dma_start(p_all[job["t"]][:, job["slz"]], pt[:])
            if job["t"] < nsteps - 1:
                # 8*p, cast to fp8 DR layout via two SWDGE partition-block copies
                pt8 = z_p.tile([A, NT], f32, tag="zpt8")
                nc.vector.tensor_scalar(pt8[:], pt[:], 8.0, None, ALU.mult)
                for j in (0, 1):
                    nc.gpsimd.dma_start(
                        job["pslot"][0:32, j, job["slz"]],
                        pt8[32 * j : 32 * (j + 1), :],
                    )

        for t in range(nsteps):
            cur, nxt = t % 2, (t + 1) % 2
            hcur, hnxt = hdr2[cur], hdr2[nxt]
            for n in range(NB):
                sl = slice(n * NT, (n + 1) * NT)
                pend = None  # deferred cell tail: (cnew, o_slice, r)

                def emit_tail(p_):
                    # tanh(c) ~ c*(a0 + a1 u + a2 u^2 + a3 u^3), u = c^2 — on DVE
                    # (keeps the ACT queue a pure eviction stream)
                    cnew_p, o_p, r_p = p_
                    a0, a1, a2, a3 = (
                        0.99848469, -0.31568862, 0.09053968, -0.01251429
                    )
                    u = cell_p.tile([128, NT], bf, tag="pu", bufs=1)
                    nc.vector.tensor_tensor(u[:], cnew_p[:], cnew_p[:], ALU.mult)
                    w1 = cell_p.tile([128, NT], bf, tag="pw1", bufs=1)
                    nc.vector.tensor_scalar(w1[:], u[:], a3, a2, ALU.mult, ALU.add)
                    w2 = cell_p.tile([128, NT], bf, tag="pw2", bufs=1)
                    nc.vector.tensor_tensor(w2[:], w1[:], u[:], ALU.mult)
                    w4 = cell_p.tile([128, NT], bf, tag="pw4", bufs=1)
                    nc.vector.scalar_tensor_tensor(
                        w4[:], w2[:], a1, u[:], ALU.add, ALU.mult
                    )
                    th = cell_p.tile([128, NT], bf, tag="pth")
                    nc.vector.scalar_tensor_tensor(
                        th[:], w4[:], a0, cnew_p[:], ALU.add, ALU.mult
                    )
                    nc.gpsimd.tensor_tensor(
                        hnxt[r_p // 2][:, r_p % 2, sl], th[:], o_p, ALU.mult
                    )

                for r in range(8):
                    # ---- rolling prefetch: one tile ahead ----
                    tt, tn = (t, n + 1) if n < NB - 1 else (t + 1, 0)
                    if tt < nsteps:
                        load_g0(tt, tn, r)
                        if tt >= 1:
                            load_cin(tt, tn, r)

                    # ---- gate blocks ----
                    g0t = g0map.pop((t, n, r))
                    gs4 = gs_p.tile([128, 4 * NT], bf, tag="gs4")
                    if t == 0:
                        for half in (0, 1):
                            nc.scalar.activation(
                                gs4[:, half * 2 * NT : (half + 1) * 2 * NT],
                                g0t[:, half * 2 * NT : (half + 1) * 2 * NT],
                                AF.Sigmoid, scale=1.0 / SCALE,
                            )
                    else:
                        for half in (0, 1):
                            ps = gps.tile([128, 2 * NT], f32, tag="g")
                            for j in (0, 1):
                                gi = 2 * half + j
                                m = gi * 8 + r
                                po = ps[:, j * NT : (j + 1) * NT]
                                first = True
                                if not ABL_NOID:
                                    nc.tensor.matmul(
                                        po, id_t[:],
                                        g0t[:, gi * NT : (gi + 1) * NT],
                                        start=True, stop=False,
                                    )
                                    first = False
                                if WBFIRST and not ABL_NOWBIG:
                                    nc.tensor.matmul(
                                        po,
                                        wb_t[:, :, m * 128 : (m + 1) * 128],
                                        p5[cur][:, :, sl],
                                        start=first, stop=False, perf_mode=DR,
                                    )
                                    first = False
                                if ABL_BF16:
                                    for k in range(8):
                                        nc.tensor.matmul(
                                            po,
                                            whbf[:, m * 128 : (m + 1) * 128],
                                            hbf[:, sl],
                                            start=first,
                                            stop=(ABL_NOWBIG and k == 7),
                                        )
                                        first = False
                                else:
                                    for k in range(4):
                                        nc.tensor.matmul(
                                            po,
                                            whdr[k][:, :, m * 128 : (m + 1) * 128],
                                            hcur[k][:, :, sl],
                                            start=first,
                                            stop=((ABL_NOWBIG or WBFIRST) and k == 3),
                                            perf_mode=DR,
                                        )
                                        first = False
                                if not ABL_NOWBIG and not WBFIRST:
                                    nc.tensor.matmul(
                                        po,
                                        wb_t[:, :, m * 128 : (m + 1) * 128],
                                        p5[cur][:, :, sl],
                                        start=False, stop=True, perf_mode=DR,
                                    )
                            nc.scalar.activation(
                                gs4[:, half * 2 * NT : (half + 1) * 2 * NT],
                                ps[:], AF.Sigmoid, scale=1.0 / SCALE,
                            )

                    # ---- deferred z pipeline (for the previous tile) ----
                    if zjob is not None:
                        if r == 1:
                            z_dr(zjob)
                        elif r == 2:
                            z_poly(zjob)
                        elif r == 4:
                            z_flush(zjob)
                            zjob = None

                    if ABL_CELL:
                        if t > 0:
                            cinmap.pop((t, n, r), None)
                        continue

                    # ---- cell update (fp32 combine on DVE) ----
                    i_s = gs4[:, 0:NT]
                    f_s = gs4[:, NT : 2 * NT]
                    graw = gs4[:, 2 * NT : 3 * NT]  # = sigmoid(2x); tanh = 2*graw-1
                    o_s = gs4[:, 3 * NT : 4 * NT]
                    t1 = cell_p.tile([128, NT], f32, tag="t1")
                    nc.vector.tensor_tensor(t1[:], graw, i_s, ALU.mult)
                    if t == 0:
                        cnew = cell_p.tile([128, NT], bf, tag="cnew", bufs=3)
                        nc.vector.scalar_tensor_tensor(
                            cnew[:], t1[:], 2.0, i_s, ALU.mult, ALU.subtract
                        )
                    else:
                        cin = cinmap.pop((t, n, r))
                        t2 = cell_p.tile([128, NT], f32, tag="t2")
                        nc.vector.scalar_tensor_tensor(
                            t2[:], t1[:], 2.0, i_s, ALU.mult, ALU.subtract
                        )
                        fc = cell_p.tile([128, NT], f32, tag="fc")
                        nc.vector.tensor_tensor(fc[:], f_s, cin[:], ALU.mult)
                        cnew = cell_p.tile([128, NT], bf, tag="cnew", bufs=3)
                        nc.vector.tensor_tensor(cnew[:], t2[:], fc[:], ALU.add)
                    if t < nsteps - 1 and not ABL_C:
                        nc.sync.dma_start(c_d[(t + 1) % 2][n, r], cnew[:])

                    if pend is not None:
                        emit_tail(pend)
                    pend = (cnew, o_s, r)
                if pend is not None:
                    emit_tail(pend)

                # queue z for this tile (consumed during the next tile)
                if not ABL_Z:
                    zjob = {
                        "t": t, "n": n, "slz": sl, "h": hnxt, "pslot": p5[nxt],
                    }

        # epilogue: flush the last z job (t = nsteps-1, n = NB-1)
        if zjob is not None:
            z_dr(zjob)
            z_poly(zjob)
            z_flush(zjob)

    nc.compile()
    return nc


# ---------------- host-side wrapper ----------------


def _prep_weights(W_xh, b_xh, W_ih, W_hh, b_ih, b_hh, W_hz, b_hz, W_emb):
    bf = ml_dtypes.bfloat16
    f8 = ml_dtypes.float8_e4m3
    f32 = np.float32
    d = {}
    # per-gate-row scale: x64 for i,f,o; x128 for g (tanh-via-sigmoid).
    rs = np.concatenate([
        np.full(H, SCALE, f32), np.full(H, SCALE, f32),
        np.full(H, 2 * SCALE, f32), np.full(H, SCALE, f32),
    ])  # [4H]
    wxh = np.zeros((KXP, H), f32)
    wxh[:E] = np.asarray(W_xh, f32).T
    d["WxhT"] = np.ascontiguousarray(wxh.reshape(3, 128, H)).astype(bf)
    d["bxh"] = np.ascontiguousarray(np.asarray(b_xh, f32).reshape(8, 128).T)
    wih = np.asarray(W_ih, f32)
    wa8 = (wih[:, :H].T * rs).reshape(4, 2, 128, G4)
    d["WihADR"] = np.ascontiguousarray(wa8.transpose(0, 2, 1, 3)).astype(f8)
    # p-term folded into a DoubleRow group: w5[P, j, m] = WbigT[j*32+P, m]*rs/8
    # for P<32 (p is stored as 8*p in fp8); zero rows for P>=32.
    wbig = wih[:, H:].astype(np.float64) @ np.asarray(W_emb, np.float64)
    wbigT = (wbig.T * rs).astype(np.float32)  # [A, G4]
    w5 = np.zeros((128, 2, G4), f32)
    w5[:32, 0, :] = wbigT[0:32] / 8.0
    w5[:32, 1, :] = wbigT[32:64] / 8.0
    d["WbigT"] = np.ascontiguousarray(w5).astype(f8)
    whh8 = (np.asarray(W_hh, f32).T * rs).reshape(4, 2, 128, G4)
    d["WhhDR"] = np.ascontiguousarray(whh8.transpose(0, 2, 1, 3)).astype(f8)
    d["bg"] = np.ascontiguousarray(
        ((np.asarray(b_ih, f32) + np.asarray(b_hh, f32)) * rs).reshape(32, 128).T
    )
    whz8 = (np.asarray(W_hz, f32).T * SCALE).reshape(4, 2, 128, A)
    d["WhzDR"] = np.ascontiguousarray(whz8.transpose(0, 2, 1, 3)).astype(f8)
    d["bhz"] = np.ascontiguousarray(np.asarray(b_hz, f32).reshape(A, 1))
    d["onesB"] = np.ones((A, 128), f32).astype(bf)
    d["ident"] = np.eye(128, dtype=f32).astype(bf)
    return d


def _prep_x(x_shard):
    bf = ml_dtypes.bfloat16
    xt = np.zeros((KXP, x_shard.shape[0]), np.float32)
    xt[:E] = np.asarray(x_shard, np.float32).T
    return np.ascontiguousarray(xt.reshape(3, 128, -1)).astype(bf)


def build_for_timing(inputs, **build_kwargs):
    wd = _prep_weights(**{k: v for k, v in inputs.items() if k != "input_x"})
    x = np.asarray(inputs["input_x"], np.float32)
    in_maps = []
    for c in range(NCORES):
        m = dict(wd)
        m["xT"] = _prep_x(x[c * BL : (c + 1) * BL])
        in_maps.append(m)
    return build_nc(**build_kwargs), in_maps


def kernel(input_x, W_xh, b_xh, W_ih, W_hh, b_ih, b_hh, W_hz, b_hz, W_emb):
    from concourse.bass_utils import run_bass_kernel_spmd

    wd = _prep_weights(W_xh, b_xh, W_ih, W_hh, b_ih, b_hh, W_hz, b_hz, W_emb)
    x = np.asarray(input_x, np.float32)
    in_maps = []
    for c in range(NCORES):
        m = dict(wd)
        m["xT"] = _prep_x(x[c * BL : (c + 1) * BL])
        in_maps.append(m)

    nc = build_nc()
    res = run_bass_kernel_spmd(nc, in_maps, list(range(NCORES)))

    out = np.empty((B, D, A), np.float32)
    for c in range(NCORES):
        pa = res.results[c]["p_all"]  # [D, A, BL]
        out[c * BL : (c + 1) * BL] = pa.transpose(2, 0, 1)
    return out, out
